# revision 26
# baseline (speedup 1.0000x reference)
"""Multi-head attention Trainium2 Bass kernel (fp8 DoubleRow edition).

Problem: B=8, N=2048, C=768, H=12 heads, D=64 head dim.
  qkv = x @ w_qkv.T          -> [B, N, 3C]
  per head: softmax(q k^T / sqrt(D)) @ v
  y = attn_out @ w_proj.T + b_proj

Sharding: data parallel over batch - one batch element per NeuronCore.

Numerics/speed strategy (all matmuls fp8e4 DoubleRow at 0.5 cy/row except
the bf16 output projection):
  - qkv: x and w_qkv*32 split hi/lo into two e4m3 tensors on the host;
    3-pass matmul (xh*wh + xh*wl + xl*wh) gives ~bf16 accuracy at fp8 cost.
  - scores: S-DoubleRow contracts d=64 twice per partition pair: sub-row 0
    carries k_hi, sub-row 1 k_lo (k effectively exact); q single e4m3
    (rhs duplicated in both sub-rows). psum = (32q)(32k) = 8192*logit.
  - softmax exp: split across engines. ScalarE tiles use the real Exp
    activation (scale 1/8192). Pool/DVE tiles use the Schraudolph trick:
    int8 code = psum*(8*log2e/8192) + 56.x, truncated; the int8 bit pattern
    IS e4m3(exp(logit)) (exponent bias 7, 3 mantissa bits). A matching
    -0.156 LSB bias on the ScalarE path keeps both flavors mean-consistent
    inside one softmax (the common factor cancels in the normalization).
  - AV: DoubleRow over key pairs; v split hi/lo (two 8-instr chains into one
    psum accumulation). Ones-column in v_hi yields the softmax denominator.
  - proj: bf16, K=128 per head pair; attention output normalized into a
    resident [128, 6, N] bf16 tile (odd heads written partition-shifted by
    the flexible gpsimd/Pool engine).
Everything is resident in SBUF (x, weights, aT) - no scratch DRAM.
"""

import numpy as np
import ml_dtypes

import concourse.bass as bass
import concourse.mybir as mybir
import concourse.tile as tile
from concourse import bacc
from concourse.bass_utils import run_bass_kernel_spmd

B, N, C, H = 8, 2048, 768, 12
D = C // H            # 64
F = 3 * C             # 2304
NT = N // 128         # 16 key tiles
NQ = 512              # query-chunk width
NCH = N // NQ         # 4 chunks
NO = 384              # proj output half-width
HP = H // 2           # 6 head pairs

FP32 = mybir.dt.float32
BF16 = mybir.dt.bfloat16
F8 = mybir.dt.float8e4
I8 = mybir.dt.int8
EXP = mybir.ActivationFunctionType.Exp
DR = mybir.MatmulPerfMode.DoubleRow

# scores psum = (32q)(32k) = 8192 * logit  (logit = q.k/sqrt(64))
EXP_SCALE = 1.0 / 8192.0
BIAS_LSB = -0.156                       # mean-match vs truncating Schraudolph
EXP_BIAS = BIAS_LSB * float(np.log(2.0)) / 8.0
SCH_A = 8.0 * float(np.log2(np.e)) / 8192.0
SCH_B = 56.0 + BIAS_LSB                 # hw rounds fp32->int8 (RTNE)

# per-unit exp engine assignment for the 8 psum groups (S=ScalarE act,
# P=Pool/gpsimd Schraudolph, V=DVE Schraudolph)
# per-unit engine for each of the 8 exp groups (2 key-tiles each):
# S=ScalarE activation, V=DVE Schraudolph. Pool cannot read PSUM.
EXP_ASSIGN = (
    ("S", "S", "S", "S", "S", "S", "S", "S"),
    ("S", "S", "S", "S", "S", "S", "S", "S"),
)

_CACHED_NC = None
_CACHED_HOST = None


def _bc_ap(dram_ap, parts):
    """Partition-broadcast a 1-D DRAM AP to [parts, len] via stride-0."""
    return bass.AP(
        tensor=dram_ap.tensor,
        offset=dram_ap.offset,
        ap=[[0, parts]] + [list(p) for p in dram_ap.ap],
    )


def build():
    nc = bacc.Bacc()
    xh = nc.dram_tensor("xh", [C, N], F8, kind="ExternalInput")
    xl = nc.dram_tensor("xl", [C, N], F8, kind="ExternalInput")
    wh = nc.dram_tensor("wh", [C, F], F8, kind="ExternalInput")
    wl = nc.dram_tensor("wl", [C, F], F8, kind="ExternalInput")
    wp = nc.dram_tensor("wp", [C, C], BF16, kind="ExternalInput")
    b_proj = nc.dram_tensor("b_proj", [C], FP32, kind="ExternalInput")
    y = nc.dram_tensor("y", [N, C], FP32, kind="ExternalOutput")

    lp = nc.allow_low_precision("fp8 attention path validated off-line")
    lp.__enter__()
    with tile.TileContext(nc) as tc:
        with tc.tile_pool(name="wpool", bufs=1) as wpool, \
             tc.tile_pool(name="apool", bufs=1) as apool, \
             tc.tile_pool(name="qk8", bufs=2) as qk8, \
             tc.tile_pool(name="epool", bufs=2) as epool, \
             tc.tile_pool(name="vpool", bufs=2) as vpool, \
             tc.tile_pool(name="small", bufs=2) as small, \
             tc.tile_pool(name="psum_fill", bufs=2, space="PSUM") as psum_fill, \
             tc.tile_pool(name="psum_s", bufs=2, space="PSUM") as psum_s, \
             tc.tile_pool(name="psum_av", bufs=2, space="PSUM") as psum_av:

            # resident inputs, split into tiles so the first qkv chains
            # start as soon as their slices land (tile-granular deps)
            HN = N // 2
            xh_t = [wpool.tile([128, 3, 2, HN], F8, tag=f"xh{c}", name=f"xh{c}")
                    for c in range(2)]
            xl_t = [wpool.tile([128, 3, 2, HN], F8, tag=f"xl{c}", name=f"xl{c}")
                    for c in range(2)]
            whqk = wpool.tile([128, 3, 2, 2 * C], F8, tag="whqk")
            wlqk = wpool.tile([128, 3, 2, 2 * C], F8, tag="wlqk")
            whv = wpool.tile([128, 3, 2, C], F8, tag="whv")
            wlv = wpool.tile([128, 3, 2, C], F8, tag="wlv")
            wp_sb = wpool.tile([128, 6, C], BF16, tag="wp")
            xr = {0: xh[:, :].rearrange("(kt two p) n -> p kt two n",
                                        p=128, two=2),
                  1: xl[:, :].rearrange("(kt two p) n -> p kt two n",
                                        p=128, two=2)}
            wr = {0: wh[:, :].rearrange("(kt two p) f -> p kt two f",
                                        p=128, two=2),
                  1: wl[:, :].rearrange("(kt two p) f -> p kt two f",
                                        p=128, two=2)}
            # order: exactly what the first q/k chains need first
            nc.sync.dma_start(out=whqk, in_=wr[0][:, :, :, 0:2 * C])
            nc.scalar.dma_start(out=xh_t[0], in_=xr[0][:, :, :, 0:HN])
            nc.sync.dma_start(out=wlqk, in_=wr[1][:, :, :, 0:2 * C])
            nc.scalar.dma_start(out=xl_t[0], in_=xr[1][:, :, :, 0:HN])
            nc.sync.dma_start(out=whv, in_=wr[0][:, :, :, 2 * C:F])
            nc.scalar.dma_start(out=xh_t[1], in_=xr[0][:, :, :, HN:N])
            nc.sync.dma_start(out=wlv, in_=wr[1][:, :, :, 2 * C:F])
            nc.scalar.dma_start(out=xl_t[1], in_=xr[1][:, :, :, HN:N])
            nc.sync.dma_start(
                out=wp_sb,
                in_=wp[:, :].rearrange("(kt p) o -> p kt o", p=128),
            )
            bias_bc = wpool.tile([128, C], FP32, tag="bias")
            nc.gpsimd.dma_start(out=bias_bc, in_=_bc_ap(b_proj[:], 128))
            aT = apool.tile([128, 6, N], BF16, tag="aT")
            ebias = wpool.tile([128, 1], FP32, tag="ebias")
            nc.vector.memset(ebias, EXP_BIAS)
            ones1 = wpool.tile([1, 128], BF16, tag="ones1")
            nc.vector.memset(ones1, 1.0)
            bias_row_f = wpool.tile([1, C], FP32, tag="biasrowf")
            nc.sync.dma_start(out=bias_row_f, in_=_bc_ap(b_proj[:], 1))
            bias_row = wpool.tile([1, C], BF16, tag="biasrow")
            nc.vector.tensor_copy(bias_row, bias_row_f)

            # per-pair state (filled by the qkv closures, read by attention)
            state = {}

            def qk_chain(hp, t, j):
                """q (t=0) or k (t=1) for pair hp, chunk j."""
                def emit():
                    qt, kt_ = state[hp]["q"], state[hp]["k"]
                    ps = psum_fill.tile([128, NQ], FP32, tag="fill")
                    fcol = t * C + hp * 128
                    xoff = (j * NQ) % HN
                    for p, (wA, xB) in enumerate(
                            ((whqk, xh_t[j // 2]), (wlqk, xh_t[j // 2]),
                             (whqk, xl_t[j // 2]))):
                        if p == 2:
                            xB = xl_t[j // 2]
                        for kt in range(3):
                            nc.tensor.matmul(
                                ps,
                                wA[:, kt, :, fcol:fcol + 128],
                                xB[:, kt, :, xoff:xoff + NQ],
                                start=(p == 0 and kt == 0),
                                stop=(p == 2 and kt == 2),
                                perf_mode=DR,
                            )
                    jsl = slice(j * NQ, (j + 1) * NQ)
                    if t == 0:
                        nc.scalar.copy(qt[:, 0, jsl], ps)
                    else:
                        nc.scalar.copy(kt_[:, 0, jsl], ps)
                        nc.vector.tensor_sub(kt_[:, 1, jsl], ps,
                                             kt_[:, 0, jsl])
                return emit

            def v_bank(hp, g):
                """v for pair hp, key tiles 4g..4g+3 (one psum bank)."""
                def emit():
                    vh0, vl0, vh1, vl1 = (state[hp][k] for k in
                                          ("vh0", "vl0", "vh1", "vl1"))
                    vps = psum_fill.tile([128, 2, 2, 128], FP32, tag="fill")
                    for b in range(4):
                        tt = 4 * g + b
                        out = vps[:, b // 2, b % 2, :]
                        xc = tt // 8
                        xo = (tt * 128) % HN
                        for p, (wA, xB) in enumerate(
                                ((whv, xh_t[xc]), (wlv, xh_t[xc]),
                                 (whv, xl_t[xc]))):
                            if p == 2:
                                xB = xl_t[xc]
                            for kt in range(3):
                                nc.tensor.matmul(
                                    out,
                                    xB[:, kt, :, xo:xo + 128],
                                    wA[:, kt, :, hp * 128:(hp + 1) * 128],
                                    start=(p == 0 and kt == 0),
                                    stop=(p == 2 and kt == 2),
                                    perf_mode=DR,
                                )
                    usl = slice(2 * g, 2 * g + 2)
                    for a, (vht, vlt) in enumerate(((vh0, vl0), (vh1, vl1))):
                        src = vps[:, :, :, a * D:(a + 1) * D]
                        nc.scalar.copy(vht[:, usl, :, 0:D], src)
                        nc.vector.tensor_sub(vlt[:, usl, :, 0:D], src,
                                             vht[:, usl, :, 0:D])
                return emit

            def make_pair_tiles(hp):
                st = {}
                st["q"] = qk8.tile([128, 1, N], F8, tag="qT8", name="qT8")
                st["k"] = qk8.tile([128, 2, N], F8, tag="kT8", name="kT8")
                for a in range(2):
                    # innermost padded to 80B: DoubleRow LdWeights requires
                    # 16B-aligned sub-row strides (s3_lw_dual_fp8)
                    vh = vpool.tile([128, NT // 2, 2, 80], F8,
                                    tag=f"vh{a}", name=f"vh{a}")
                    vl = vpool.tile([128, NT // 2, 2, 80], F8,
                                    tag=f"vl{a}", name=f"vl{a}")
                    nc.gpsimd.memset(vh[:, :, :, D:D + 1], 1.0)
                    nc.gpsimd.memset(vl[:, :, :, D:D + 1], 0.0)
                    st[f"vh{a}"], st[f"vl{a}"] = vh, vl
                state[hp] = st

            def qkv_closures(hp):
                make_pair_tiles(hp)
                cl = []
                for j in range(NCH):
                    cl.append(qk_chain(hp, 0, j))
                    cl.append(qk_chain(hp, 1, j))
                for g in range(4):
                    cl.append(v_bank(hp, g))
                return cl

            def proj_chain(i, half):
                def emit():
                    pp2 = psum_fill.tile([128, NQ], FP32, tag="fill")
                    pp = pp2[:, 0:NO]
                    nc.tensor.matmul(
                        pp, ones1,
                        bias_row[:, half * NO:(half + 1) * NO],
                        start=True, stop=False,
                    )
                    for kt in range(6):
                        nc.tensor.matmul(
                            pp,
                            aT[:, kt, i * 128:(i + 1) * 128],
                            wp_sb[:, kt, half * NO:(half + 1) * NO],
                            start=False,
                            stop=(kt == 5),
                        )
                    ys = small.tile([128, NO], FP32, tag="ys", bufs=2)
                    nc.scalar.copy(ys, pp)
                    nc.sync.dma_start(
                        out=y[i * 128:(i + 1) * 128,
                              half * NO:(half + 1) * NO],
                        in_=ys,
                    )
                return emit

            # ---------------- main schedule ----------------
            for emit in qkv_closures(0):
                emit()

            filler = []
            unit_idx = 0
            prev = None   # (av, vht, vlt, expS, hp, jsl, a) of previous unit

            def emit_av_pair(pu, g):
                av_, vht_, vlt_, expS_, _, _, _ = pu
                # hi instr for group g, then lo instr for group g
                nc.tensor.matmul(
                    av_, vht_[:, g, :, 0:D + 1], expS_[:, g, :, :],
                    start=(g == 0), stop=False, perf_mode=DR,
                    skip_group_check=True,
                )
                nc.tensor.matmul(
                    av_, vlt_[:, g, :, 0:D + 1], expS_[:, g, :, :],
                    start=False, stop=(g == 7), perf_mode=DR,
                    skip_group_check=True,
                )

            def emit_norm(pu):
                av_, _, _, _, hp_, jsl_, a_ = pu
                rec = small.tile([1, NQ], FP32, tag="rec")
                nc.vector.reciprocal(rec, av_[D:D + 1, :])
                bct = small.tile([D, NQ], FP32, tag="bct")
                nc.gpsimd.partition_broadcast(bct, rec)
                avt = small.tile([D, NQ], BF16, tag="avt")
                nc.scalar.copy(avt, av_[0:D, :])
                nc.gpsimd.tensor_mul(
                    aT[a_ * D:(a_ + 1) * D, hp_, jsl_], avt, bct)

            NF = (2, 2, 2, 2, 1, 1, 1, 1)
            for hp in range(HP):
                if hp < HP - 1:
                    filler.extend(qkv_closures(hp + 1))
                for j in range(NCH):
                    jsl = slice(j * NQ, (j + 1) * NQ)
                    for a in range(2):
                        st = state[hp]
                        qt, kt_ = st["q"], st["k"]
                        vht = st[f"vh{a}"]
                        vlt = st[f"vl{a}"]
                        lo = a * D
                        expS = epool.tile([128, NT // 2, 2, NQ], F8,
                                          tag="expS", name="expS")
                        assign = EXP_ASSIGN[unit_idx % 2]
                        qrhs = qt[lo:lo + D, 0:1, jsl].to_broadcast(
                            [D, 2, NQ])
                        av = psum_av.tile([D + 1, NQ], FP32, tag="av")
                        nfill = 3 if hp == HP - 1 else NF[unit_idx % 8]
                        for u in range(8):
                            sps = psum_s.tile([128, 2, NQ], FP32, tag="sps",
                                              name="sps")
                            for i2 in range(2):
                                t = 2 * u + i2
                                nc.tensor.matmul(
                                    sps[:, i2, :],
                                    kt_[lo:lo + D, :, t * 128:(t + 1) * 128],
                                    qrhs,
                                    start=True,
                                    stop=True,
                                    perf_mode=DR,
                                )
                            eout = expS[:, u, :, :]
                            if assign[u] == "S":
                                nc.scalar.activation(
                                    out=eout,
                                    in_=sps,
                                    func=EXP,
                                    scale=EXP_SCALE,
                                    bias=ebias[:, :],
                                )
                            else:
                                nc.vector.tensor_scalar(
                                    eout.bitcast(I8),
                                    sps,
                                    SCH_A,
                                    SCH_B,
                                    mybir.AluOpType.mult,
                                    mybir.AluOpType.add,
                                )
                            if prev is not None:
                                emit_av_pair(prev, u)
                            if u < nfill and filler:
                                filler.pop(0)()
                        if prev is not None:
                            emit_norm(prev)
                        prev = (av, vht, vlt, expS, hp, jsl, a)
                        unit_idx += 1
                        # chunk jj's aT is complete once norm of (hp5, jj, 1)
                        # has been emitted, i.e. during unit (hp5, jj+1, 0)
                        if hp == HP - 1 and a == 0 and j > 0:
                            for i in range(4 * (j - 1), 4 * j):
                                filler.append(proj_chain(i, 0))
                                filler.append(proj_chain(i, 1))
            for u in range(8):
                emit_av_pair(prev, u)
            emit_norm(prev)
            for i in range(4 * (NCH - 1), 4 * NCH):
                filler.append(proj_chain(i, 0))
                filler.append(proj_chain(i, 1))
            for emit in filler:
                emit()
    lp.__exit__(None, None, None)

    nc.finalize()
    return nc


def get_nc():
    global _CACHED_NC
    if _CACHED_NC is None:
        _CACHED_NC = build()
    return _CACHED_NC


LAST_RESULT = None


def _host_prep(x, w_qkv, w_proj, b_proj):
    E4 = ml_dtypes.float8_e4m3
    w32T = np.ascontiguousarray((w_qkv * 32.0).T.astype(np.float32))
    whh = w32T.astype(E4)
    wll = (w32T - whh.astype(np.float32)).astype(E4)
    wpT = np.ascontiguousarray((w_proj.T / 32.0).astype(ml_dtypes.bfloat16))
    maps = []
    for i in range(B):
        xT = np.ascontiguousarray(x[i].T)
        xhh = xT.astype(E4)
        xll = (xT - xhh.astype(np.float32)).astype(E4)
        maps.append({
            "xh": xhh, "xl": xll, "wh": whh, "wl": wll,
            "wp": wpT, "b_proj": b_proj,
        })
    return maps


def kernel(x, w_qkv, w_proj, b_proj, **run_kwargs):
    x = np.ascontiguousarray(np.asarray(x, dtype=np.float32))
    w_qkv = np.ascontiguousarray(np.asarray(w_qkv, dtype=np.float32))
    w_proj = np.ascontiguousarray(np.asarray(w_proj, dtype=np.float32))
    b_proj = np.ascontiguousarray(np.asarray(b_proj, dtype=np.float32))
    assert x.shape == (B, N, C)

    nc = get_nc()
    in_maps = _host_prep(x, w_qkv, w_proj, b_proj)
    res = run_bass_kernel_spmd(nc, in_maps, list(range(B)), **run_kwargs)
    global LAST_RESULT
    LAST_RESULT = res
    out = np.stack([res.results[i]["y"] for i in range(B)], axis=0)
    return out


if __name__ == "__main__":
    rng = np.random.default_rng(0)
    x = rng.standard_normal((B, N, C), dtype=np.float32)
    w_qkv = (rng.standard_normal((F, C)) * 0.02).astype(np.float32)
    w_proj = (rng.standard_normal((C, C)) * 0.02).astype(np.float32)
    b_proj = (rng.standard_normal((C,)) * 0.02).astype(np.float32)
    out = kernel(x=x, w_qkv=w_qkv, w_proj=w_proj, b_proj=b_proj)
    print("out", out.shape, out.dtype, float(np.abs(out).max()))


# revision 27
# speedup vs baseline: 1.4088x; 1.4088x over previous
"""Multi-head attention Trainium2 Bass kernel (fp8 DoubleRow edition).

Problem: B=8, N=2048, C=768, H=12 heads, D=64 head dim.
  qkv = x @ w_qkv.T          -> [B, N, 3C]
  per head: softmax(q k^T / sqrt(D)) @ v
  y = attn_out @ w_proj.T + b_proj

Sharding: data parallel over batch - one batch element per NeuronCore.

Numerics/speed strategy (all matmuls fp8e4 DoubleRow at 0.5 cy/row except
the bf16 output projection):
  - qkv: x and w_qkv*32 split hi/lo into two e4m3 tensors on the host;
    3-pass matmul (xh*wh + xh*wl + xl*wh) gives ~bf16 accuracy at fp8 cost.
  - scores: S-DoubleRow contracts d=64 twice per partition pair: sub-row 0
    carries k_hi, sub-row 1 k_lo (k effectively exact); q single e4m3
    (rhs duplicated in both sub-rows). psum = (32q)(32k) = 8192*logit.
  - softmax exp: split across engines. ScalarE tiles use the real Exp
    activation (scale 1/8192). Pool/DVE tiles use the Schraudolph trick:
    int8 code = psum*(8*log2e/8192) + 56.x, truncated; the int8 bit pattern
    IS e4m3(exp(logit)) (exponent bias 7, 3 mantissa bits). A matching
    -0.156 LSB bias on the ScalarE path keeps both flavors mean-consistent
    inside one softmax (the common factor cancels in the normalization).
  - AV: DoubleRow over key pairs; v split hi/lo (two 8-instr chains into one
    psum accumulation). Ones-column in v_hi yields the softmax denominator.
  - proj: bf16, K=128 per head pair; attention output normalized into a
    resident [128, 6, N] bf16 tile (odd heads written partition-shifted by
    the flexible gpsimd/Pool engine).
Everything is resident in SBUF (x, weights, aT) - no scratch DRAM.
"""

import numpy as np
import ml_dtypes

import concourse.bass as bass
import concourse.mybir as mybir
import concourse.tile as tile
from concourse import bacc
from concourse.bass_utils import run_bass_kernel_spmd

B, N, C, H = 8, 2048, 768, 12
D = C // H            # 64
F = 3 * C             # 2304
NT = N // 128         # 16 key tiles
NQ = 512              # query-chunk width
NCH = N // NQ         # 4 chunks
NO = 384              # proj output half-width
HP = H // 2           # 6 head pairs

FP32 = mybir.dt.float32
BF16 = mybir.dt.bfloat16
F8 = mybir.dt.float8e4
I8 = mybir.dt.int8
EXP = mybir.ActivationFunctionType.Exp
DR = mybir.MatmulPerfMode.DoubleRow

# scores psum = (32q)(32k) = 8192 * logit  (logit = q.k/sqrt(64))
EXP_SCALE = 1.0 / 8192.0
BIAS_LSB = 0.0
EXP_BIAS = BIAS_LSB * float(np.log(2.0)) / 8.0
SCH_A = 8.0 * float(np.log2(np.e)) / 8192.0
# hw fp32->int8 is RTNE; -0.344 cancels the mean of the linear-mantissa
# interpolation distortion so both exp flavors are mean-ideal
SCH_B = 56.0 - 0.344

# per-unit exp engine assignment for the 8 psum groups (S=ScalarE act,
# P=Pool/gpsimd Schraudolph, V=DVE Schraudolph)
# per-unit engine for each of the 8 exp groups (2 key-tiles each):
# S=ScalarE activation, V=DVE Schraudolph. Pool cannot read PSUM.
EXP_ASSIGN = (
    ("S", "V", "S", "V", "S", "V", "S", "V"),
    ("V", "S", "V", "S", "V", "S", "V", "S"),
)

_CACHED_NC = None
_CACHED_HOST = None


def _bc_ap(dram_ap, parts):
    """Partition-broadcast a 1-D DRAM AP to [parts, len] via stride-0."""
    return bass.AP(
        tensor=dram_ap.tensor,
        offset=dram_ap.offset,
        ap=[[0, parts]] + [list(p) for p in dram_ap.ap],
    )


def build():
    nc = bacc.Bacc()
    xh = nc.dram_tensor("xh", [C, N], F8, kind="ExternalInput")
    xl = nc.dram_tensor("xl", [C, N], F8, kind="ExternalInput")
    wh = nc.dram_tensor("wh", [C, F], F8, kind="ExternalInput")
    wl = nc.dram_tensor("wl", [C, F], F8, kind="ExternalInput")
    wp = nc.dram_tensor("wp", [C, C], BF16, kind="ExternalInput")
    b_proj = nc.dram_tensor("b_proj", [C], FP32, kind="ExternalInput")
    y = nc.dram_tensor("y", [N, C], FP32, kind="ExternalOutput")

    lp = nc.allow_low_precision("fp8 attention path validated off-line")
    lp.__enter__()
    with tile.TileContext(nc) as tc:
        with tc.tile_pool(name="wpool", bufs=1) as wpool, \
             tc.tile_pool(name="apool", bufs=1) as apool, \
             tc.tile_pool(name="qk8", bufs=2) as qk8, \
             tc.tile_pool(name="epool", bufs=2) as epool, \
             tc.tile_pool(name="vpool", bufs=2) as vpool, \
             tc.tile_pool(name="small", bufs=2) as small, \
             tc.tile_pool(name="psum_fill", bufs=2, space="PSUM") as psum_fill, \
             tc.tile_pool(name="psum_s", bufs=2, space="PSUM") as psum_s, \
             tc.tile_pool(name="psum_av", bufs=2, space="PSUM") as psum_av:

            # resident inputs, split into tiles so the first qkv chains
            # start as soon as their slices land (tile-granular deps)
            HN = N // 2
            xh_t = [wpool.tile([128, 3, 2, HN], F8, tag=f"xh{c}", name=f"xh{c}")
                    for c in range(2)]
            xl_t = [wpool.tile([128, 3, 2, HN], F8, tag=f"xl{c}", name=f"xl{c}")
                    for c in range(2)]
            whqk = wpool.tile([128, 3, 2, 2 * C], F8, tag="whqk")
            wlqk = wpool.tile([128, 3, 2, 2 * C], F8, tag="wlqk")
            whv = wpool.tile([128, 3, 2, C], F8, tag="whv")
            wlv = wpool.tile([128, 3, 2, C], F8, tag="wlv")
            wp_sb = wpool.tile([128, 6, C], BF16, tag="wp")
            xr = {0: xh[:, :].rearrange("(kt two p) n -> p kt two n",
                                        p=128, two=2),
                  1: xl[:, :].rearrange("(kt two p) n -> p kt two n",
                                        p=128, two=2)}
            wr = {0: wh[:, :].rearrange("(kt two p) f -> p kt two f",
                                        p=128, two=2),
                  1: wl[:, :].rearrange("(kt two p) f -> p kt two f",
                                        p=128, two=2)}
            # order: exactly what the first q/k chains need first
            nc.sync.dma_start(out=whqk, in_=wr[0][:, :, :, 0:2 * C])
            nc.scalar.dma_start(out=xh_t[0], in_=xr[0][:, :, :, 0:HN])
            nc.sync.dma_start(out=wlqk, in_=wr[1][:, :, :, 0:2 * C])
            nc.scalar.dma_start(out=xl_t[0], in_=xr[1][:, :, :, 0:HN])
            nc.sync.dma_start(out=whv, in_=wr[0][:, :, :, 2 * C:F])
            nc.scalar.dma_start(out=xh_t[1], in_=xr[0][:, :, :, HN:N])
            nc.sync.dma_start(out=wlv, in_=wr[1][:, :, :, 2 * C:F])
            nc.scalar.dma_start(out=xl_t[1], in_=xr[1][:, :, :, HN:N])
            nc.sync.dma_start(
                out=wp_sb,
                in_=wp[:, :].rearrange("(kt p) o -> p kt o", p=128),
            )
            bias_bc = wpool.tile([128, C], FP32, tag="bias")
            nc.gpsimd.dma_start(out=bias_bc, in_=_bc_ap(b_proj[:], 128))
            aT = apool.tile([128, 6, N], BF16, tag="aT")
            ebias = wpool.tile([128, 1], FP32, tag="ebias")
            nc.vector.memset(ebias, EXP_BIAS)
            ones1 = wpool.tile([1, 128], BF16, tag="ones1")
            nc.vector.memset(ones1, 1.0)
            bias_row_f = wpool.tile([1, C], FP32, tag="biasrowf")
            nc.sync.dma_start(out=bias_row_f, in_=_bc_ap(b_proj[:], 1))
            bias_row = wpool.tile([1, C], BF16, tag="biasrow")
            nc.vector.tensor_copy(bias_row, bias_row_f)

            # per-pair state (filled by the qkv closures, read by attention)
            state = {}

            def qk_chain(hp, t, j):
                """q (t=0) or k (t=1) for pair hp, chunk j."""
                def emit():
                    qt, kt_ = state[hp]["q"], state[hp]["k"]
                    ps = psum_fill.tile([128, NQ], FP32, tag="fill")
                    fcol = t * C + hp * 128
                    xoff = (j * NQ) % HN
                    for p, (wA, xB) in enumerate(
                            ((whqk, xh_t[j // 2]), (wlqk, xh_t[j // 2]),
                             (whqk, xl_t[j // 2]))):
                        if p == 2:
                            xB = xl_t[j // 2]
                        for kt in range(3):
                            nc.tensor.matmul(
                                ps,
                                wA[:, kt, :, fcol:fcol + 128],
                                xB[:, kt, :, xoff:xoff + NQ],
                                start=(p == 0 and kt == 0),
                                stop=(p == 2 and kt == 2),
                                perf_mode=DR,
                            )
                    jsl = slice(j * NQ, (j + 1) * NQ)
                    if t == 0:
                        nc.scalar.copy(qt[:, 0, jsl], ps)
                    else:
                        nc.scalar.copy(kt_[:, 0, jsl], ps)
                        nc.vector.tensor_sub(kt_[:, 1, jsl], ps,
                                             kt_[:, 0, jsl])
                return emit

            def v_bank(hp, g):
                """v for pair hp, key tiles 4g..4g+3 (one psum bank)."""
                def emit():
                    vh0, vl0, vh1, vl1 = (state[hp][k] for k in
                                          ("vh0", "vl0", "vh1", "vl1"))
                    vps = psum_fill.tile([128, 2, 2, 128], FP32, tag="fill")
                    for b in range(4):
                        tt = 4 * g + b
                        out = vps[:, b // 2, b % 2, :]
                        xc = tt // 8
                        xo = (tt * 128) % HN
                        for p, (wA, xB) in enumerate(
                                ((whv, xh_t[xc]), (wlv, xh_t[xc]),
                                 (whv, xl_t[xc]))):
                            if p == 2:
                                xB = xl_t[xc]
                            for kt in range(3):
                                nc.tensor.matmul(
                                    out,
                                    xB[:, kt, :, xo:xo + 128],
                                    wA[:, kt, :, hp * 128:(hp + 1) * 128],
                                    start=(p == 0 and kt == 0),
                                    stop=(p == 2 and kt == 2),
                                    perf_mode=DR,
                                )
                    usl = slice(2 * g, 2 * g + 2)
                    for a, (vht, vlt) in enumerate(((vh0, vl0), (vh1, vl1))):
                        src = vps[:, :, :, a * D:(a + 1) * D]
                        nc.scalar.copy(vht[:, usl, :, 0:D], src)
                        nc.vector.tensor_sub(vlt[:, usl, :, 0:D], src,
                                             vht[:, usl, :, 0:D])
                return emit

            def make_pair_tiles(hp):
                st = {}
                st["q"] = qk8.tile([128, 1, N], F8, tag="qT8", name="qT8")
                st["k"] = qk8.tile([128, 2, N], F8, tag="kT8", name="kT8")
                for a in range(2):
                    # innermost padded to 80B: DoubleRow LdWeights requires
                    # 16B-aligned sub-row strides (s3_lw_dual_fp8)
                    vh = vpool.tile([128, NT // 2, 2, 80], F8,
                                    tag=f"vh{a}", name=f"vh{a}")
                    vl = vpool.tile([128, NT // 2, 2, 80], F8,
                                    tag=f"vl{a}", name=f"vl{a}")
                    nc.gpsimd.memset(vh[:, :, :, D:D + 1], 1.0)
                    nc.gpsimd.memset(vl[:, :, :, D:D + 1], 0.0)
                    st[f"vh{a}"], st[f"vl{a}"] = vh, vl
                state[hp] = st

            def qkv_closures(hp):
                make_pair_tiles(hp)
                cl = []
                for j in range(NCH):
                    cl.append(qk_chain(hp, 0, j))
                    cl.append(qk_chain(hp, 1, j))
                for g in range(4):
                    cl.append(v_bank(hp, g))
                return cl

            def proj_chain(i, half):
                def emit():
                    pp2 = psum_fill.tile([128, NQ], FP32, tag="fill")
                    pp = pp2[:, 0:NO]
                    nc.tensor.matmul(
                        pp, ones1,
                        bias_row[:, half * NO:(half + 1) * NO],
                        start=True, stop=False,
                    )
                    for kt in range(6):
                        nc.tensor.matmul(
                            pp,
                            aT[:, kt, i * 128:(i + 1) * 128],
                            wp_sb[:, kt, half * NO:(half + 1) * NO],
                            start=False,
                            stop=(kt == 5),
                        )
                    ys = small.tile([128, NO], FP32, tag="ys", bufs=2)
                    nc.scalar.copy(ys, pp)
                    nc.sync.dma_start(
                        out=y[i * 128:(i + 1) * 128,
                              half * NO:(half + 1) * NO],
                        in_=ys,
                    )
                return emit

            # ---------------- main schedule ----------------
            for emit in qkv_closures(0):
                emit()

            filler = []
            unit_idx = 0
            prev = None   # (av, vht, vlt, expS, hp, jsl, a) of previous unit

            def emit_av_pair(pu, g):
                av_, vht_, vlt_, expS_, _, _, _ = pu
                # hi instr for group g, then lo instr for group g
                nc.tensor.matmul(
                    av_, vht_[:, g, :, 0:D + 1], expS_[:, g, :, :],
                    start=(g == 0), stop=False, perf_mode=DR,
                    skip_group_check=True,
                )
                nc.tensor.matmul(
                    av_, vlt_[:, g, :, 0:D + 1], expS_[:, g, :, :],
                    start=False, stop=(g == 7), perf_mode=DR,
                    skip_group_check=True,
                )

            def emit_norm(pu):
                av_, _, _, _, hp_, jsl_, a_ = pu
                rec = small.tile([1, NQ], FP32, tag="rec")
                nc.vector.reciprocal(rec, av_[D:D + 1, :])
                bct = small.tile([D, NQ], FP32, tag="bct")
                nc.gpsimd.partition_broadcast(bct, rec)
                avt = small.tile([D, NQ], BF16, tag="avt")
                nc.scalar.copy(avt, av_[0:D, :])
                nc.gpsimd.tensor_mul(
                    aT[a_ * D:(a_ + 1) * D, hp_, jsl_], avt, bct)

            NF = (2, 2, 2, 2, 1, 1, 1, 1)
            for hp in range(HP):
                if hp < HP - 1:
                    filler.extend(qkv_closures(hp + 1))
                for j in range(NCH):
                    jsl = slice(j * NQ, (j + 1) * NQ)
                    for a in range(2):
                        st = state[hp]
                        qt, kt_ = st["q"], st["k"]
                        vht = st[f"vh{a}"]
                        vlt = st[f"vl{a}"]
                        lo = a * D
                        expS = epool.tile([128, NT // 2, 2, NQ], F8,
                                          tag="expS", name="expS")
                        assign = EXP_ASSIGN[unit_idx % 2]
                        qrhs = qt[lo:lo + D, 0:1, jsl].to_broadcast(
                            [D, 2, NQ])
                        av = psum_av.tile([D + 1, NQ], FP32, tag="av")
                        nfill = 3 if hp == HP - 1 else NF[unit_idx % 8]
                        for u in range(8):
                            sps = psum_s.tile([128, 2, NQ], FP32, tag="sps",
                                              name="sps")
                            for i2 in range(2):
                                t = 2 * u + i2
                                nc.tensor.matmul(
                                    sps[:, i2, :],
                                    kt_[lo:lo + D, :, t * 128:(t + 1) * 128],
                                    qrhs,
                                    start=True,
                                    stop=True,
                                    perf_mode=DR,
                                )
                            eout = expS[:, u, :, :]
                            if assign[u] == "S":
                                nc.scalar.activation(
                                    out=eout,
                                    in_=sps,
                                    func=EXP,
                                    scale=EXP_SCALE,
                                    bias=ebias[:, :],
                                )
                            else:
                                nc.vector.tensor_scalar(
                                    eout.bitcast(I8),
                                    sps,
                                    SCH_A,
                                    SCH_B,
                                    mybir.AluOpType.mult,
                                    mybir.AluOpType.add,
                                )
                            if prev is not None:
                                emit_av_pair(prev, u)
                            if u < nfill and filler:
                                filler.pop(0)()
                        if prev is not None:
                            emit_norm(prev)
                        prev = (av, vht, vlt, expS, hp, jsl, a)
                        unit_idx += 1
                        # chunk jj's aT is complete once norm of (hp5, jj, 1)
                        # has been emitted, i.e. during unit (hp5, jj+1, 0)
                        if hp == HP - 1 and a == 0 and j > 0:
                            for i in range(4 * (j - 1), 4 * j):
                                filler.append(proj_chain(i, 0))
                                filler.append(proj_chain(i, 1))
            for u in range(8):
                emit_av_pair(prev, u)
            emit_norm(prev)
            for i in range(4 * (NCH - 1), 4 * NCH):
                filler.append(proj_chain(i, 0))
                filler.append(proj_chain(i, 1))
            for emit in filler:
                emit()
    lp.__exit__(None, None, None)

    nc.finalize()
    return nc


def get_nc():
    global _CACHED_NC
    if _CACHED_NC is None:
        _CACHED_NC = build()
    return _CACHED_NC


LAST_RESULT = None


def _host_prep(x, w_qkv, w_proj, b_proj):
    E4 = ml_dtypes.float8_e4m3
    w32T = np.ascontiguousarray((w_qkv * 32.0).T.astype(np.float32))
    whh = w32T.astype(E4)
    wll = (w32T - whh.astype(np.float32)).astype(E4)
    wpT = np.ascontiguousarray((w_proj.T / 32.0).astype(ml_dtypes.bfloat16))
    maps = []
    for i in range(B):
        xT = np.ascontiguousarray(x[i].T)
        xhh = xT.astype(E4)
        xll = (xT - xhh.astype(np.float32)).astype(E4)
        maps.append({
            "xh": xhh, "xl": xll, "wh": whh, "wl": wll,
            "wp": wpT, "b_proj": b_proj,
        })
    return maps


def kernel(x, w_qkv, w_proj, b_proj, **run_kwargs):
    x = np.ascontiguousarray(np.asarray(x, dtype=np.float32))
    w_qkv = np.ascontiguousarray(np.asarray(w_qkv, dtype=np.float32))
    w_proj = np.ascontiguousarray(np.asarray(w_proj, dtype=np.float32))
    b_proj = np.ascontiguousarray(np.asarray(b_proj, dtype=np.float32))
    assert x.shape == (B, N, C)

    nc = get_nc()
    in_maps = _host_prep(x, w_qkv, w_proj, b_proj)
    res = run_bass_kernel_spmd(nc, in_maps, list(range(B)), **run_kwargs)
    global LAST_RESULT
    LAST_RESULT = res
    out = np.stack([res.results[i]["y"] for i in range(B)], axis=0)
    return out


if __name__ == "__main__":
    rng = np.random.default_rng(0)
    x = rng.standard_normal((B, N, C), dtype=np.float32)
    w_qkv = (rng.standard_normal((F, C)) * 0.02).astype(np.float32)
    w_proj = (rng.standard_normal((C, C)) * 0.02).astype(np.float32)
    b_proj = (rng.standard_normal((C,)) * 0.02).astype(np.float32)
    out = kernel(x=x, w_qkv=w_qkv, w_proj=w_proj, b_proj=b_proj)
    print("out", out.shape, out.dtype, float(np.abs(out).max()))
